# revision 33
# baseline (speedup 1.0000x reference)
"""Trainium2 Bass kernel for nn_MultiHeadSelfAttention_29403346108551.

Reference semantics (faithful to the original nn.Module):
  q/k/v = (x @ W.T + b) .reshape(b, 16, 2048, 64)   # reshape, NOT transpose
  RoPE with a *scalar* position t=seq_len (same angle for every token),
  scores = q k^T / 8, softmax, o = p v, merge heads, o @ wo.T + bo.

Structural facts used for sharding:
  - The head split is a row-major reshape: head h reads x rows [128h, 128h+128)
    and ALL 1024 features; within-head time t = r*16 + jc (r = x-row in block,
    jc = feature chunk j//64), d = j%64.  Permuted time t'' = jc*128 + r is
    used on-device; the host un-permutes.
  - RoPE rotation folded into wq/wk/bq/bk on the host (scalar position).
  - Core cid: batch cid//4, head group cid%4 (4 heads = x rows [512g, 512g+512)).
    Output projection partials summed across the 4 cores of a batch on host.

Design (v6):
  - All matmul operands bf16 (PSUM accumulation f32); rel-err ~7.7e-3 measured.
  - Every matmul is full-array tile_size (128,128): scores use a zero-padded
    interleaved Q layout (A-half: qA in partitions 0-63 / zeros 64-127;
    B-half mirrored) so one [128,128] kT2a chunk (kA rows 0-63, kB rows
    64-127) serves both heads of a pair.  No PE tiling-mode switches.
  - exp runs entirely on ACT (~1.0us per [128,1024] tile == the PE's per-iter
    matmul time; a 1-op DVE Schraudolph exp measured 1.9e-2 end-to-end and
    was dropped).  Softmax denominators via the ones-column in v_aug.
  - m-outer iteration: the pair-1 projections (N=256 matmuls) are deferred
    and interleaved into the pair-0 attention stream, hiding them under the
    ACT exp cadence.
  - Output projection accumulates BOTH head pairs into one PSUM tile;
    host sums 4 cores per batch and un-permutes t''.
  - Tail: the last group's softmax-normalize uses a PE broadcast matmul for
    1/den instead of the gpsimd partition_broadcast (whose dma<->broadcast
    ucode library switch costs ~3us).
"""

import numpy as np
import ml_dtypes

import concourse.bass as bass
import concourse.mybir as mybir
import concourse.tile as tile
from concourse import bacc
from concourse.bass_utils import run_bass_kernel_spmd

F32 = mybir.dt.float32
F32R = mybir.dt.float32r
BF16 = mybir.dt.bfloat16
I16 = mybir.dt.int16

MODEL_DIM = 1024
NUM_HEADS = 16
D_K = 64
B = 2
T = 2048
N_CORES = 8
NK = 8              # contraction chunks of 128 over MODEL_DIM
RPC = 512           # x rows per core
SEQ_POS = 2048      # scalar rope position used by the reference


def _build_program() -> bass.Bass:
    nc = bacc.Bacc(None, target_bir_lowering=False, debug=False)

    xT = nc.dram_tensor("xT", [MODEL_DIM, RPC], BF16, kind="ExternalInput")
    wqT = nc.dram_tensor("wqT", [MODEL_DIM, MODEL_DIM], BF16, kind="ExternalInput")
    wkT = nc.dram_tensor("wkT", [MODEL_DIM, MODEL_DIM], BF16, kind="ExternalInput")
    wvT = nc.dram_tensor("wvT", [MODEL_DIM, MODEL_DIM], BF16, kind="ExternalInput")
    woT = nc.dram_tensor("woT", [2, 128, MODEL_DIM], BF16, kind="ExternalInput")
    bq = nc.dram_tensor("bq", [128, 8], F32, kind="ExternalInput")
    bk = nc.dram_tensor("bk", [128, 8], F32, kind="ExternalInput")
    bv = nc.dram_tensor("bv", [MODEL_DIM], F32, kind="ExternalInput")
    outp = nc.dram_tensor("outp", [T, MODEL_DIM], F32, kind="ExternalOutput")

    with tile.TileContext(nc) as tc:
        with (
            tc.tile_pool(name="xpool", bufs=8) as xpool,
            tc.tile_pool(name="wpool", bufs=24) as wpool,
            tc.tile_pool(name="cpool", bufs=1) as cpool,
            tc.tile_pool(name="qkpool", bufs=1) as qkpool,
            tc.tile_pool(name="vpool", bufs=4) as vpool,
            tc.tile_pool(name="espool", bufs=4) as espool,
            tc.tile_pool(name="o2pool", bufs=1) as o2pool,
            tc.tile_pool(name="outpool", bufs=4) as outpool,
            tc.tile_pool(name="opool", bufs=2) as opool,
            tc.tile_pool(name="rcpool", bufs=2) as rcpool,
            tc.tile_pool(name="rcbig", bufs=2) as rcbig,
        ):
            # ---- warmup MMs keep the PE busy (HAM warm) during input DMA ----
            warm_w = cpool.tile([128, 512], BF16, name="warm_w")
            nc.vector.memset(warm_w, 0.0)
            ones64 = cpool.tile([1, 64], F32R, name="ones64")
            nc.vector.memset(ones64.bitcast(F32), 1.0)
            with tc.tile_pool(name="pswarm", bufs=1, space="PSUM") as pswarm:
                psw = pswarm.tile([128, 512], F32, name="psw")
                for i in range(18):
                    nc.tensor.matmul(psw, warm_w[:, 0:128], warm_w,
                                     start=True, stop=True)
                wsink = cpool.tile([1, 16], F32, name="wsink")
                nc.scalar.activation(wsink, psw[0:1, 0:16],
                                     mybir.ActivationFunctionType.Exp, scale=1.0)

            # ---- input loads (xt + wq first: first proj MM needs them) ----
            xt = []
            for k in range(NK):
                t_ = xpool.tile([128, RPC], BF16, tag="xt", name=f"xt_{k}")
                nc.sync.dma_start(out=t_, in_=xT[k * 128:(k + 1) * 128, :])
                xt.append(t_)
            wq_sb, wk_sb = [], []
            for k in range(NK):
                t_ = wpool.tile([128, MODEL_DIM], BF16, tag="w", name=f"wq_{k}")
                eng = nc.scalar if k % 2 == 0 else nc.gpsimd
                eng.dma_start(out=t_, in_=wqT[k * 128:(k + 1) * 128, :])
                wq_sb.append(t_)
            for k in range(NK):
                t_ = wpool.tile([128, MODEL_DIM], BF16, tag="w", name=f"wk_{k}")
                nc.sync.dma_start(out=t_, in_=wkT[k * 128:(k + 1) * 128, :])
                wk_sb.append(t_)

            bq_sb = cpool.tile([128, 8], F32)
            nc.sync.dma_start(out=bq_sb, in_=bq[:, :])
            bk_sb = cpool.tile([128, 8], F32)
            nc.sync.dma_start(out=bk_sb, in_=bk[:, :])
            bv_bc = cpool.tile([128, MODEL_DIM], F32)
            nc.sync.dma_start(
                out=bv_bc,
                in_=bass.AP(tensor=bv[:].tensor, offset=bv[:].offset,
                            ap=[[0, 128]] + [list(p) for p in bv[:].ap]))

            # persistent activation layouts
            # qT2pad: [pair m (2), wq (4), {A,B} halves (2), 512] columns, bf16.
            #   A half: qA d-vals in partitions 0-63, zeros 64-127; B mirrored.
            qT2pad = qkpool.tile([128, 2 * 4 * 2 * 512], BF16, name="qT2pad")
            nc.vector.memset(qT2pad, 0.0)
            # kT2a: pair m at cols [m*T, (m+1)*T); partition = 64*parity + d;
            #   col = t'' = jc*128 + r
            kT2a = qkpool.tile([128, 2 * T], BF16, name="kT2a")
            o2T = [o2pool.tile([128, T], BF16, name=f"o2T_{i}") for i in range(2)]
            wo_sb = []
            for m_ in range(2):
                t_ = cpool.tile([128, MODEL_DIM], BF16, name=f"wo_{m_}")
                nc.scalar.dma_start(out=t_, in_=woT[m_, :, :])
                wo_sb.append(t_)

            # v_aug[bl]: [128 keys(r), 16 chunks(jc), 65]; col 64 = ones
            v_aug = []
            for bl in range(4):
                va = vpool.tile([128, 16, 65], BF16, tag="va", name=f"v_aug_{bl}")
                nc.vector.memset(va[:, :, 64:65], 1.0)
                v_aug.append(va)

            # preload ACT exp table set + gpsimd dma/broadcast libraries early
            warm2 = cpool.tile([1, 16], F32, name="warm2")
            nc.scalar.activation(warm2, warm_w.bitcast(F32)[0:1, 0:16],
                                 mybir.ActivationFunctionType.Exp, scale=1.0)
            gwa = cpool.tile([1, 16], F32, name="gwa")
            gwb = cpool.tile([4, 16], F32, name="gwb")
            nc.gpsimd.dma_start(out=gwa, in_=warm_w.bitcast(F32)[0:1, 0:16])
            nc.gpsimd.partition_broadcast(gwb, gwa)
            nc.gpsimd.dma_start(out=gwa, in_=gwb[0:1, :])
            nc.gpsimd.partition_broadcast(gwb, gwa)

            def drain_qk(ps, p, is_q, m, bias_sb, use_act=False):
                """Drain psq [128, 256] (pair-m half) into qT2pad/kT2a.
                use_act puts half the ops on the (idle in phase A) ACT engine
                via Identity-with-bias."""
                for half in range(2):
                    jc = 2 * p + half
                    for ph in range(2):
                        src = ps[64 * half:64 * half + 64,
                                 ph * 128:ph * 128 + 128]
                        if is_q:
                            base = (m * 8 + 2 * (jc // 4) + ph) * 512 \
                                + (jc % 4) * 128
                            dst = qT2pad[64 * ph:64 * ph + 64,
                                         base:base + 128]
                        else:
                            base = m * T + jc * 128
                            dst = kT2a[64 * ph:64 * ph + 64, base:base + 128]
                        bias_ap = bias_sb[64 * half:64 * half + 64, p:p + 1]
                        if use_act and ph == 0:
                            nc.scalar.activation(
                                dst, src,
                                mybir.ActivationFunctionType.Identity,
                                bias=bias_ap, scale=1.0)
                        else:
                            nc.vector.tensor_scalar_add(dst, src, bias_ap)

            def drain_v(ps, bl, jw):
                nc.vector.tensor_tensor(
                    v_aug[bl][:, 8 * jw:8 * jw + 8, 0:64],
                    ps[:, :].rearrange("p (cc d) -> p cc d", d=64),
                    bv_bc[:, jw * 512:(jw + 1) * 512].rearrange(
                        "p (cc d) -> p cc d", d=64),
                    mybir.AluOpType.add)

            # ---- phase A: pair-0 projections (N=256 matmuls) ----
            with tc.tile_pool(name="psproj", bufs=8, space="PSUM") as psproj:
                wv_sb = []
                for k in range(NK):
                    t_ = wpool.tile([128, MODEL_DIM], BF16, tag="w", name=f"wv_{k}")
                    nc.scalar.dma_start(out=t_, in_=wvT[k * 128:(k + 1) * 128, :])
                    wv_sb.append(t_)

                for w_sb, bias_sb, is_q in ((wq_sb, bq_sb, True),
                                            (wk_sb, bk_sb, False)):
                    for p in range(8):
                        ps = psproj.tile([128, 256], F32, tag="proj",
                                         name=f"ps0_{int(is_q)}_{p}")
                        for k in range(NK):
                            nc.tensor.matmul(
                                ps, w_sb[k][:, p * 128:(p + 1) * 128],
                                xt[k][:, 0:256],
                                start=(k == 0), stop=(k == NK - 1))
                        drain_qk(ps, p, is_q, 0, bias_sb, use_act=True)
                for bl in range(2):
                    for jw in range(2):
                        ps = psproj.tile([128, RPC], F32, tag="proj",
                                         name=f"psv0_{bl}_{jw}")
                        for k in range(NK):
                            nc.tensor.matmul(
                                ps, xt[k][:, bl * 128:(bl + 1) * 128],
                                wv_sb[k][:, jw * 512:(jw + 1) * 512],
                                start=(k == 0), stop=(k == NK - 1))
                        drain_v(ps, bl, jw)

            # ---- deferred pair-1 projection emission (phase B interleave) ----
            def gen_deferred(pool):
                for w_sb, bias_sb, is_q in ((wk_sb, bk_sb, False),
                                            (wq_sb, bq_sb, True)):
                    for p in range(8):
                        ps = pool.tile([128, 256], F32, tag="pq2", bufs=2,
                                       name=f"ps1_{int(is_q)}_{p}")
                        for k in range(NK):
                            nc.tensor.matmul(
                                ps, w_sb[k][:, p * 128:(p + 1) * 128],
                                xt[k][:, 256:512],
                                start=(k == 0), stop=(k == NK - 1))
                            yield
                        drain_qk(ps, p, is_q, 1, bias_sb)
                for bl in range(2, 4):
                    for jw in range(2):
                        ps = pool.tile([128, RPC], F32, tag="pq2", bufs=2,
                                       name=f"psv1_{bl}_{jw}")
                        for k in range(NK):
                            nc.tensor.matmul(
                                ps, xt[k][:, bl * 128:(bl + 1) * 128],
                                wv_sb[k][:, jw * 512:(jw + 1) * 512],
                                start=(k == 0), stop=(k == NK - 1))
                            yield
                        drain_v(ps, bl, jw)

            # ---- attention ----
            with (
                tc.tile_pool(name="psS", bufs=2, space="PSUM") as psS_pool,
                tc.tile_pool(name="psO", bufs=1, space="PSUM") as psO_pool,
            ):
                state = {}

                def emit_scores(i, wq, m, cc):
                    psS = psS_pool.tile([128, 1024], F32, tag="s", bufs=2,
                                        name=f"psS_{i}")
                    lhs = kT2a[:, m * T + cc * 128: m * T + cc * 128 + 128]
                    base = (m * 4 + wq) * 1024
                    nc.tensor.matmul(psS[:, 0:512], lhs,
                                     qT2pad[:, base:base + 512],
                                     start=True, stop=True)
                    nc.tensor.matmul(psS[:, 512:1024], lhs,
                                     qT2pad[:, base + 512:base + 1024],
                                     start=True, stop=True)
                    eS = espool.tile([128, 1024], I16, tag="es", bufs=4,
                                     name=f"eS_{i}")
                    nc.scalar.activation(
                        eS.bitcast(BF16), psS,
                        mybir.ActivationFunctionType.Exp, scale=0.125)
                    return eS

                def normalize_pair(m, wq, psO_A, psO_B, tail_ctx=None):
                    """psO_{A,B} [65,512] -> o2T[m][:, wq*512:+512].
                    Mid-kernel: gpsimd broadcast (hidden under the PE stream).
                    Tail (tail_ctx = psF pool): PE broadcast matmul instead --
                    the gpsimd dma->broadcast library switch costs ~3us."""
                    o_sbs, rcps = [], []
                    for ph, psO in ((0, psO_A), (1, psO_B)):
                        o_sb = opool.tile([65, 512], F32, tag=f"osb{ph}",
                                          bufs=2, name=f"osb{ph}_{m}_{wq}")
                        nc.vector.tensor_copy(o_sb, psO)
                        o_sbs.append(o_sb)
                    for ph in range(2):
                        den_t = rcpool.tile([128, 4], F32, tag=f"dent{ph}",
                                            bufs=2)
                        nc.gpsimd.dma_start(
                            out=den_t,
                            in_=o_sbs[ph][64:65, :].rearrange(
                                "a (p i) -> a p i", p=128))
                        rcp_t = rcpool.tile([128, 4], F32, tag=f"rcpt{ph}",
                                            bufs=2)
                        nc.vector.reciprocal(rcp_t, den_t)
                        rcps.append(rcp_t)
                    flats = []
                    for ph in range(2):
                        rcp_flat = rcbig.tile([1, 512], F32, tag=f"rcpf{ph}",
                                              bufs=2)
                        nc.gpsimd.dma_start(
                            out=rcp_flat[0:1, :].rearrange(
                                "a (p i) -> a p i", p=128),
                            in_=rcps[ph])
                        flats.append(rcp_flat)
                    for ph in range(2):
                        if tail_ctx is None:
                            rcp_bc = rcbig.tile([64, 512], F32,
                                                tag=f"rcpb{ph}", bufs=2)
                            nc.gpsimd.partition_broadcast(rcp_bc, flats[ph])
                        else:
                            rcp_bc = tail_ctx.tile([64, 512], F32,
                                                   tag=f"f{ph}",
                                                   name=f"psRc{ph}")
                            nc.tensor.matmul(rcp_bc, ones64,
                                             flats[ph].bitcast(F32R),
                                             start=True, stop=True)
                        nc.vector.tensor_tensor(
                            o2T[m][64 * ph:64 * ph + 64,
                                   wq * 512:(wq + 1) * 512],
                            o_sbs[ph][0:64, :], rcp_bc, mybir.AluOpType.mult)

                def emit_pv(i, wq, m, cc, eS):
                    if cc == 0:
                        state[(wq, m)] = (
                            psO_pool.tile([65, 512], F32, tag="oA",
                                          name=f"psO_A_{wq}_{m}"),
                            psO_pool.tile([65, 512], F32, tag="oB",
                                          name=f"psO_B_{wq}_{m}"),
                        )
                    psO_A, psO_B = state[(wq, m)]
                    eSb = eS.bitcast(BF16)
                    nc.tensor.matmul(psO_A, v_aug[2 * m][:, cc, :],
                                     eSb[:, 0:512],
                                     start=(cc == 0), stop=(cc == 15))
                    nc.tensor.matmul(psO_B, v_aug[2 * m + 1][:, cc, :],
                                     eSb[:, 512:1024],
                                     start=(cc == 0), stop=(cc == 15))
                    if cc == 15 and not (wq == 3 and m == 1):
                        normalize_pair(m, wq, psO_A, psO_B)

                # ---- phase B: pair-0 attention with deferred projections ----
                pend = []
                with tc.tile_pool(name="psq2", bufs=2, space="PSUM") as psq2:
                    dgen = gen_deferred(psq2)
                    demitted, dtotal = 0, 160
                    for j in range(64):
                        wq, cc = j // 16, j % 16
                        eS = emit_scores(j, wq, 0, cc)
                        pend.append((j, wq, 0, cc, eS))
                        if len(pend) > 2:
                            emit_pv(*pend.pop(0))
                        if pend and pend[0][3] == 15:
                            emit_pv(*pend.pop(0))
                        # interleave deferred pair-1 projection matmuls
                        want = (j + 1) * dtotal // 64
                        while demitted < want:
                            if next(dgen, None) is None:
                                demitted = dtotal
                                break
                            demitted += 1
                    for _ in range(len(pend)):
                        emit_pv(*pend.pop(0))
                    for _ in dgen:
                        pass

                # ---- phase C: pair-1 attention + finals ----
                with tc.tile_pool(name="psF", bufs=1, space="PSUM") as psF_pool:

                    def emit_final(tt, tail=False):
                        psh = [psF_pool.tile([128, 512], F32, tag=f"f{j}",
                                             name=f"psF_{tt}_{j}")
                               for j in range(2)]
                        for jw in range(2):
                            for m2 in range(2):
                                nc.tensor.matmul(
                                    psh[jw],
                                    o2T[m2][:, tt * 128:(tt + 1) * 128],
                                    wo_sb[m2][:, jw * 512:(jw + 1) * 512],
                                    start=(m2 == 0), stop=(m2 == 1))
                        out_sb = outpool.tile([128, MODEL_DIM], F32,
                                              tag="out", bufs=4)
                        if tail:
                            # ACT is idle at the tail: parallel copies + eager
                            # half DMAs shorten the drain
                            nc.scalar.copy(out_sb[:, 0:512], psh[0])
                            nc.sync.dma_start(
                                out=outp[tt * 128:(tt + 1) * 128, 0:512],
                                in_=out_sb[:, 0:512])
                            nc.vector.tensor_copy(out_sb[:, 512:1024], psh[1])
                            nc.sync.dma_start(
                                out=outp[tt * 128:(tt + 1) * 128, 512:1024],
                                in_=out_sb[:, 512:1024])
                        else:
                            nc.vector.tensor_copy(out_sb[:, 0:512], psh[0])
                            nc.vector.tensor_copy(out_sb[:, 512:1024], psh[1])
                            nc.sync.dma_start(
                                out=outp[tt * 128:(tt + 1) * 128, :],
                                in_=out_sb)

                    nfin = 0
                    for j in range(64):
                        wq, cc = j // 16, j % 16
                        i = 64 + j
                        eS = emit_scores(i, wq, 1, cc)
                        pend.append((i, wq, 1, cc, eS))
                        if len(pend) > 2:
                            emit_pv(*pend.pop(0))
                        if pend and pend[0][3] == 15:
                            emit_pv(*pend.pop(0))
                        # finals(wq-1) need o2T[1] group wq-1 (done early this
                        # wq) and o2T[0] (phase B); finals(3) come after wq=3
                        if wq >= 1 and ((cc in (6, 10, 14)) or cc == 2):
                            emit_final(4 * (wq - 1) + (nfin % 4))
                            nfin += 1
                    for _ in range(len(pend)):
                        emit_pv(*pend.pop(0))
                    # dummy matmuls (with an ACT consumer so the scheduler
                    # keeps them in place) bridge the normalize-chain latency
                    # and keep the HAM clock warm for the trailing finals
                    psd = psF_pool.tile([128, 512], F32, tag="f0", name="psd")
                    for _ in range(16):
                        nc.tensor.matmul(psd, warm_w[:, 0:128], warm_w,
                                         start=True, stop=True)
                    sink2 = cpool.tile([1, 16], F32, name="sink2")
                    nc.scalar.activation(sink2, psd[0:1, 0:16],
                                         mybir.ActivationFunctionType.Identity,
                                         scale=1.0)
                    normalize_pair(1, 3, *state[(3, 1)], tail_ctx=psF_pool)
                    for tt2 in range(12, 16):
                        emit_final(tt2, tail=True)

    nc.compile()
    return nc


_NC_CACHE = None


def _get_program():
    global _NC_CACHE
    if _NC_CACHE is None:
        _NC_CACHE = _build_program()
    return _NC_CACHE


def _bf16(a: np.ndarray) -> np.ndarray:
    return np.asarray(a, np.float32).astype(ml_dtypes.bfloat16)


def _host_prep(inputs):
    x = np.asarray(inputs["x"], np.float32)
    wq = np.asarray(inputs["wq"], np.float32)
    wk = np.asarray(inputs["wk"], np.float32)
    wv = np.asarray(inputs["wv"], np.float32)
    wo = np.asarray(inputs["wo"], np.float32)
    bq = np.asarray(inputs["bq"], np.float32)
    bk = np.asarray(inputs["bk"], np.float32)
    bv = np.asarray(inputs["bv"], np.float32)
    rot_cos = np.asarray(inputs["rot_cos"], np.float32)
    rot_sin = np.asarray(inputs["rot_sin"], np.float32)

    cos = rot_cos[SEQ_POS]
    sin = rot_sin[SEQ_POS]

    def rope_fold_w(w):
        wv_ = w.reshape(16, 32, 2, MODEL_DIM)
        ev = wv_[:, :, 0] * cos[None, :, None] - wv_[:, :, 1] * sin[None, :, None]
        od = wv_[:, :, 0] * sin[None, :, None] + wv_[:, :, 1] * cos[None, :, None]
        return np.stack([ev, od], axis=2).reshape(MODEL_DIM, MODEL_DIM)

    def rope_fold_b(b_):
        bv_ = b_.reshape(16, 32, 2)
        ev = bv_[:, :, 0] * cos - bv_[:, :, 1] * sin
        od = bv_[:, :, 0] * sin + bv_[:, :, 1] * cos
        return np.stack([ev, od], axis=2).reshape(MODEL_DIM)

    wq_r = rope_fold_w(wq)
    wk_r = rope_fold_w(wk)
    bq_r = rope_fold_b(bq)
    bk_r = rope_fold_b(bk)

    wqT = _bf16(np.ascontiguousarray(wq_r.T))
    wkT = _bf16(np.ascontiguousarray(wk_r.T))
    wvT = _bf16(np.ascontiguousarray(wv.T))
    bq_sb = np.ascontiguousarray(bq_r.reshape(8, 128).T)
    bk_sb = np.ascontiguousarray(bk_r.reshape(8, 128).T)

    in_maps = []
    for cid in range(N_CORES):
        bi, g = cid // 4, cid % 4
        xTc = _bf16(np.ascontiguousarray(x[bi, 512 * g:512 * (g + 1), :].T))
        woTc = np.stack(
            [np.ascontiguousarray(
                wo[:, (4 * g + 2 * m) * 64:(4 * g + 2 * m + 2) * 64].T)
             for m in range(2)])
        in_maps.append({
            "xT": xTc,
            "wqT": wqT, "wkT": wkT, "wvT": wvT,
            "woT": _bf16(woTc),
            "bq": bq_sb, "bk": bk_sb, "bv": bv,
        })
    return in_maps, np.asarray(inputs["bo"], np.float32)


def _gather(results, bo):
    out = np.empty((B, T, MODEL_DIM), np.float32)
    for bi in range(B):
        acc = results[4 * bi]["outp"].astype(np.float32)
        for g in range(1, 4):
            acc = acc + results[4 * bi + g]["outp"]
        # t'' = jc*128 + r  ->  t = r*16 + jc
        acc = acc.reshape(16, 128, MODEL_DIM).transpose(1, 0, 2).reshape(
            T, MODEL_DIM)
        out[bi] = acc + bo[None, :]
    return out


def _run(inputs, trace=False, **kw):
    nc = _get_program()
    in_maps, bo = _host_prep(inputs)
    res = run_bass_kernel_spmd(nc, in_maps, list(range(N_CORES)), trace=trace,
                               **kw)
    return _gather(res.results, bo), res


def kernel(**inputs) -> np.ndarray:
    out, _ = _run(inputs)
    return out
